# revision 1
# baseline (speedup 1.0000x reference)
"""Trainium2 Bass kernel for ConditionalFeedForward (MoE routed FFN).

Problem: M=2048 tokens, D=1024, I=2048, E=8 experts, TOPK=2.
out[t, s] = FFN_{e}(x[t]) with e = expert_indices[t, s], where
FFN_e(x) = (silu(x @ w1_e.T) * (x @ w3_e.T)) @ w2_e.T  (w13 = [w1; w3]).

Strategy (expert parallelism, 8 experts -> 8 cores):
 - Host routes (token, slot) pairs to the core owning the selected expert,
   pads each core's token batch to a common capacity C, and transposes
   activations so features live on SBUF partitions and tokens on the free
   dim.  No device collectives needed: the "all-to-all" is a host gather
   and scatter around one SPMD kernel launch.
 - Everything on the PE path is bf16 (PE streams 1 col/cycle for both bf16
   and fp32r, but bf16 halves HBM traffic: 12 MB of weights per core fits
   under the PE time with slack, so the whole weight set is prefetched
   into SBUF and the PE never stalls mid-stream).  PSUM accumulation is
   fp32; silu*gate runs on ACT+DVE in fp32 and requantizes g to bf16.
 - Startup: the kernel's critical payload (all of x plus pair-0's weights)
   is packed into one partition-major DRAM tensor and moved as two large
   DMAs on the sync HWDGE ring (measured ~420 GB/s; the scalar ring only
   manages ~70 GB/s and is left unused).  Big per-partition runs matter:
   small/fragmented transfers run at a fraction of peak while the DMA path
   ramps.  Pair 0 interleaves its psA/psB k-loops so the PE keeps pace
   with the arriving x stream.
 - PE warmup: the HAM clock gate holds the PE at 1.2 GHz until it has
   been busy ~3.4us, and data cannot arrive before ~12us (queue prologue +
   DMA latency), so ~4.3us of scratch matmuls on memset zeros run first;
   the clock is at 2.4 GHz by the time real matmuls start.
 - Tail: the last output d-block is processed in two column chunks so the
   final PSUM->SBUF copy + DMA-out mostly overlaps the preceding matmuls.

Measured on the staged inputs: ~98-99 us HW exec (baseline 114.9 us),
rel l2 err 3.7e-3 vs fp64 (gate 2e-2).  Steady-state is PE-bound at the
1 col/cycle bf16 stream rate: 384 matmuls x 497 cols = 79.5 us, plus
~7.2 us fixed queue prologue, ~5 us of DMA-critical startup, ~5 us tail.
"""

import os

import numpy as np
import ml_dtypes

import concourse.bass as bass
import concourse.tile as tile
from concourse import bacc, mybir
from concourse.bass_utils import run_bass_kernel_spmd

M, D, I, E, TOPK = 2048, 1024, 2048, 8, 2
P = 128
KD = D // P            # 8   k-tiles over D (mm1 contraction)
NI2 = (2 * I) // P     # 32  n-tiles over 2I (mm1 output rows)
NPAIR = NI2 // 2       # 16  (x1, x3) pairs
KI = I // P            # 16  k-tiles over I (mm2 contraction)
ND = D // P            # 8   d-tiles over D (mm2 output rows)

F32 = mybir.dt.float32
BF16 = mybir.dt.bfloat16
NP_BF16 = ml_dtypes.bfloat16

# exec time of the most recent launch (ns), populated when BASS_TRACE=1
LAST_EXEC_TIME_NS = None

_program_cache = {}


def _chunks_for(C):
    """Split C token-columns into matmul moving-dim chunks (<=512 each)."""
    n_ch = -(-C // 512)
    base = -(-C // (n_ch * 32)) * 32
    chunks = []
    off = 0
    while off < C:
        cn = min(base, C - off)
        chunks.append((off, cn))
        off += cn
    return tuple(chunks)


def _build_program(C):
    chunks = _chunks_for(C)
    nc = bacc.Bacc(
        "TRN2",
        target_bir_lowering=False,
        debug=False,
        enable_asserts=False,
        num_devices=E,
    )

    # xw0: startup-critical payload packed into one partition-major tensor
    # so it moves as TWO large-element DMAs (big contiguous runs per
    # partition are what the DMA engines stream fastest, especially during
    # the first ~6us while the DMA path is still ramping up):
    #   [ x k0-3 | wA0 k0-3 | wB0 k0-3 | x k4-7 | wA0 k4-7 | wB0 k4-7 ]
    # w13: (x1, x3) row-block PAIRS fused per partition row (pairs 1..15)
    # w2: two d-blocks fused per row
    XW = KD * C + 2 * KD * P
    H1 = 4 * C + KD * P          # end of piece 1
    xw0_d = nc.dram_tensor("xw0", (P, XW), BF16, kind="ExternalInput").ap()
    w13_d = nc.dram_tensor(
        "w13t", (NPAIR - 1, P, 2 * KD * P), BF16, kind="ExternalInput"
    ).ap()
    w2_d = nc.dram_tensor(
        "w2t", (ND // 2, P, 2 * KI * P), BF16, kind="ExternalInput"
    ).ap()
    out_d = nc.dram_tensor("outT", (ND, P, C), F32, kind="ExternalOutput").ap()

    with tile.TileContext(nc) as tc:
        with (
            tc.tile_pool(name="xg", bufs=1) as xg_pool,
            tc.tile_pool(name="wt", bufs=1) as w_pool,
            tc.tile_pool(name="tmp", bufs=4) as tmp_pool,
            tc.tile_pool(name="ps", bufs=8, space="PSUM") as ps_pool,
        ):
            # ---- PE warmup: dummy matmuls on (uninitialized) SBUF ----
            # The HAM clock gate holds the PE at 1.2 GHz until it has been
            # busy ~3.4us.  Real data can't arrive before ~10us (queue
            # prologue + DMA first-byte latency + DMA-path ramp), so burn
            # that window on scratch matmuls: the PE hits 2.4 GHz right as
            # real matmuls start.  Results land in scratch PSUM, never read.
            # Warmup needs >=3.4us of SUSTAINED matmul activity to fire the
            # HAM clock-gate (10 x ~0.43us cold matmuls); after that the PE
            # may sit idle up to ~3.4us (one MID window) without being
            # re-throttled, which comfortably covers the gap until the
            # first x/w piece lands (~12.5-14us).
            zt = xg_pool.tile([P, 256], BF16, tag="zt", name="zt")
            nc.vector.memset(zt[:], 0.0)
            for _ in range(20):
                psw = ps_pool.tile([P, 256], F32, tag="ps", name="ps")
                nc.tensor.matmul(
                    psw, zt[:, :P], zt[:], start=True, stop=True
                )

            # ---- all DMAs issued upfront, most-urgent first ----
            # The sync-queue HWDGE ring measures ~420 GB/s once streaming;
            # the scalar-queue ring only ~70 GB/s, so EVERYTHING goes on
            # sync, in exactly need-order.
            xw0 = xg_pool.tile([P, XW], BF16, tag="x", name="x")

            def x_tile(k):
                off = k * C + (KD * P if k >= 4 else 0)
                return xw0[:, off : off + C]

            def w0_tile(half, k):
                # half 0 -> wA0 slice, half 1 -> wB0 slice, for k-tile k
                off = (8 * C + KD * P if k >= 4 else 4 * C) + half * (
                    KD * P // 2
                ) + (k % 4) * P
                return xw0[:, off : off + P]

            x_tiles = [x_tile(k) for k in range(KD)]
            w13s = {
                pr: w_pool.tile(
                    [P, 2 * KD * P], BF16, tag=f"w13_{pr}", name=f"w13_{pr}"
                )
                for pr in range(1, NPAIR)
            }
            w2s = [
                w_pool.tile(
                    [P, 2 * KI * P], BF16, tag=f"w2_{j}", name=f"w2_{j}"
                )
                for j in range(ND // 2)
            ]

            nc.sync.dma_start(xw0[:, :H1], xw0_d[:, :H1])
            nc.sync.dma_start(xw0[:, H1:], xw0_d[:, H1:])
            for pr in range(1, NPAIR):
                nc.sync.dma_start(w13s[pr][:], w13_d[pr - 1])
            for j in range(ND // 2):
                nc.sync.dma_start(w2s[j][:], w2_d[j])

            g_tiles = [
                xg_pool.tile([P, C], BF16, tag=f"g{ki}", name=f"g{ki}")
                for ki in range(KI)
            ]

            # ---- mm1 + silu*gate: process (x1, x3) row-block pairs ----
            # pair 0 interleaves the psA/psB k-loops so each arriving x
            # k-tile feeds two matmuls back-to-back (PE keeps pace with the
            # x stream instead of stalling then bursting).
            for pr in range(NPAIR):
                if pr == 0:
                    lhsT = w0_tile
                else:
                    slab = w13s[pr]
                    lhsT = lambda half, k, slab=slab: slab[
                        :, half * KD * P + k * P : half * KD * P + (k + 1) * P
                    ]
                for c0, cn in chunks:
                    psA = ps_pool.tile([P, 512], F32, tag="ps", name="ps")[:, :cn]
                    psB = ps_pool.tile([P, 512], F32, tag="ps", name="ps")[:, :cn]
                    if pr == 0:
                        for k in range(KD):
                            for half, ps_ in ((0, psA), (1, psB)):
                                nc.tensor.matmul(
                                    ps_,
                                    lhsT(half, k),
                                    x_tiles[k][:, c0 : c0 + cn],
                                    start=(k == 0),
                                    stop=(k == KD - 1),
                                )
                    else:
                        for half, ps_ in ((0, psA), (1, psB)):
                            for k in range(KD):
                                nc.tensor.matmul(
                                    ps_,
                                    lhsT(half, k),
                                    x_tiles[k][:, c0 : c0 + cn],
                                    start=(k == 0),
                                    stop=(k == KD - 1),
                                )
                    s = tmp_pool.tile([P, 512], F32, tag="s", name="s")[:, :cn]
                    nc.scalar.activation(s, psA, mybir.ActivationFunctionType.Silu)
                    nc.vector.tensor_mul(
                        out=g_tiles[pr][:, c0 : c0 + cn],
                        in0=s,
                        in1=psB,
                    )

            # ---- mm2: outT[d-block] = sum_ki w2T-tile @ g ----
            # last d-block runs in two half-chunks so its copy+DMA-out
            # overlaps compute instead of sitting in the kernel tail.
            for d in range(ND):
                wDD = w2s[d // 2]
                wD = wDD[:, (d % 2) * KI * P : (d % 2 + 1) * KI * P]
                if d == ND - 1 and len(chunks) == 1 and C >= 256:
                    out_chunks = [(0, C - 64), (C - 64, 64)]
                else:
                    out_chunks = chunks
                for c0, cn in out_chunks:
                    psO = ps_pool.tile([P, 512], F32, tag="ps", name="ps")[:, :cn]
                    for ki in range(KI):
                        nc.tensor.matmul(
                            psO,
                            wD[:, ki * P : (ki + 1) * P],
                            g_tiles[ki][:, c0 : c0 + cn],
                            start=(ki == 0),
                            stop=(ki == KI - 1),
                        )
                    ot = tmp_pool.tile([P, 512], F32, tag="o", name="o")[:, :cn]
                    nc.vector.tensor_copy(ot, psO)
                    nc.sync.dma_start(out_d[d][:, c0 : c0 + cn], ot)

    nc.compile()
    return nc


def _get_program(C):
    if C not in _program_cache:
        _program_cache[C] = _build_program(C)
    return _program_cache[C]


def _ensure_ntff_hook():
    """Provide antenv.axon_hooks if the image lacks it, so trace=True works."""
    import sys
    import types

    try:
        import antenv.axon_hooks  # noqa: F401

        return
    except ImportError:
        pass
    try:
        import antenv
        from trn_agent_boot.trn_boot import _ntff_profile_via_ctypes

        mod = types.ModuleType("antenv.axon_hooks")
        state = {"hook": None}
        mod.set_axon_ntff_profile_hook = lambda h: state.__setitem__("hook", h)
        mod.get_axon_ntff_profile_hook = lambda: state["hook"]
        sys.modules["antenv.axon_hooks"] = mod
        antenv.axon_hooks = mod
        mod.set_axon_ntff_profile_hook(
            _ntff_profile_via_ctypes("/opt/axon/libaxon_pjrt.so")
        )
    except Exception:
        pass


def kernel(x, w13, w2, expert_indices):
    global LAST_EXEC_TIME_NS
    x = np.asarray(x, dtype=np.float32)
    w13 = np.asarray(w13, dtype=np.float32)
    w2 = np.asarray(w2, dtype=np.float32)
    idx = np.asarray(expert_indices)
    idx32 = idx.astype(np.int64)

    m, d_model = x.shape
    e, two_i, _ = w13.shape
    inter = w2.shape[2]
    topk = idx.shape[1]
    assert (m, d_model, e, two_i, inter, topk) == (M, D, E, 2 * I, I, TOPK)

    # ---- host routing: unique (token, expert) work items per expert ----
    # A token picking the same expert in both slots computes the FFN once;
    # the result is scattered to every matching slot.
    tok_unique = [
        np.unique(np.concatenate([np.nonzero(idx32[:, s] == ei)[0] for s in range(topk)]))
        for ei in range(E)
    ]
    max_cnt = max(len(u) for u in tok_unique)
    C = max(256, int(max_cnt))

    nc = _get_program(C)

    in_maps = []
    for ei in range(E):
        tok_ids = tok_unique[ei]
        cnt = len(tok_ids)

        xg = np.zeros((C, D), dtype=np.float32)
        xg[:cnt] = x[tok_ids]
        xT = np.ascontiguousarray(
            xg.T.reshape(KD, P, C).transpose(1, 0, 2).astype(NP_BF16)
        )                                            # [p, k, c]

        A4 = w13[ei].astype(NP_BF16).reshape(NI2, P, KD, P)   # [n, c, k, p]
        w13t = A4.transpose(0, 3, 2, 1).reshape(NI2, P, KD * P)
        w13p = np.ascontiguousarray(
            np.concatenate([w13t[:NPAIR], w13t[NPAIR:]], axis=2)
        )                                            # [pair, p, 2*KD*P]

        # pair 0 rides with x in the packed startup tensor:
        # [ x k0-3 | wA0 k0-3 | wB0 k0-3 | x k4-7 | wA0 k4-7 | wB0 k4-7 ]
        H = KD * P // 2
        xw0 = np.concatenate(
            [
                xT[:, :4].reshape(P, 4 * C),
                w13p[0][:, :H],                      # wA0 k0-3
                w13p[0][:, KD * P : KD * P + H],     # wB0 k0-3
                xT[:, 4:].reshape(P, 4 * C),
                w13p[0][:, H : KD * P],              # wA0 k4-7
                w13p[0][:, KD * P + H :],            # wB0 k4-7
            ],
            axis=1,
        )

        B4 = w2[ei].astype(NP_BF16).reshape(ND, P, KI, P)     # [d, c, ki, p]
        w2t = B4.transpose(0, 3, 2, 1).reshape(ND, P, KI * P)
        w2p = np.ascontiguousarray(
            w2t.reshape(ND // 2, 2, P, KI * P).transpose(0, 2, 1, 3).reshape(
                ND // 2, P, 2 * KI * P
            )
        )                                            # [dpair, p, 2*KI*P]

        in_maps.append({"xw0": xw0, "w13t": w13p[1:], "w2t": w2p})

    trace = bool(os.environ.get("BASS_TRACE"))
    if trace:
        _ensure_ntff_hook()
    res = run_bass_kernel_spmd(nc, in_maps, core_ids=list(range(E)), trace=trace)
    LAST_EXEC_TIME_NS = res.exec_time_ns

    # ---- host scatter: copy each expert's outputs to all matching slots ----
    out = np.empty((M, topk, D), dtype=np.float32)
    for ei in range(E):
        outT = res.results[ei]["outT"].reshape(D, C)
        oe = outT[:, : len(tok_unique[ei])].T        # [cnt, D]
        for s in range(topk):
            sel = np.nonzero(idx32[:, s] == ei)[0]
            out[sel, s] = oe[np.searchsorted(tok_unique[ei], sel)]

    return out



# revision 2
# speedup vs baseline: 1.0020x; 1.0020x over previous
"""Trainium2 Bass kernel for ConditionalFeedForward (MoE routed FFN).

Problem: M=2048 tokens, D=1024, I=2048, E=8 experts, TOPK=2.
out[t, s] = FFN_{e}(x[t]) with e = expert_indices[t, s], where
FFN_e(x) = (silu(x @ w1_e.T) * (x @ w3_e.T)) @ w2_e.T  (w13 = [w1; w3]).

Strategy (expert parallelism, 8 experts -> 8 cores):
 - Host routes (token, slot) pairs to the core owning the selected expert,
   pads each core's token batch to a common capacity C, and transposes
   activations so features live on SBUF partitions and tokens on the free
   dim.  No device collectives needed: the "all-to-all" is a host gather
   and scatter around one SPMD kernel launch.
 - Everything on the PE path is bf16 (PE streams 1 col/cycle for both bf16
   and fp32r, but bf16 halves HBM traffic).  PSUM accumulation is fp32;
   silu*gate runs on ACT+DVE in fp32 and requantizes g to bf16.  fp8 was
   measured at 5.9% rel err end-to-end (gate 2e-2) - dead on arrival.
 - Startup: the critical payload is split into 8 per-k PIECES, each one
   contiguous DRAM tensor slice [x_k | wA0_k | wB0_k | wA1_k | wB1_k], so
   the very first piece (~258 KB) completes ~1.3 us earlier than the old
   two-half layout and pair-0/1 matmuls start as soon as piece 0 lands.
   Pair-2's w13 slab is issued in the middle of the piece train so pair 2
   never stalls.  All input DMAs ride the sync HWDGE ring in need-order.
 - PE warmup: the HAM clock gate holds the PE at 1.2 GHz until it has
   been busy ~3.4us; scratch matmuls on a gpsimd-memset tile run from
   ~6.3us (right after the framework preamble) until real data lands.
 - Tail: the last output d-block runs in two column chunks, and the out
   DMAs alternate between the sync and scalar HWDGE rings so the final
   chunk's DMA never queues behind a predecessor on the same ring.

Measured on the staged inputs: ~95-96 us HW exec (baseline 114.9 us,
previous best 98.7 us), rel l2 err 3.7e-3 vs fp64 (gate 2e-2).
Breakdown: ~80.5us PE stream at the bf16 1 col/cycle roofline (384
matmuls x 497 cols) + ~1us cold-clock penalty + ~2.6us data-arrival
head + ~2us out tail + ~9.5us fixed framework preamble/teardown (the
per-semaphore reset train at end-of-NEFF is paid even by an empty
kernel - measured 14.5us exec for a trivial 2-DMA program).
"""

import os

import numpy as np
import ml_dtypes

import concourse.bass as bass
import concourse.tile as tile
from concourse import bacc, mybir
from concourse.bass_utils import run_bass_kernel_spmd

M, D, I, E, TOPK = 2048, 1024, 2048, 8, 2
P = 128
KD = D // P            # 8   k-tiles over D (mm1 contraction)
NI2 = (2 * I) // P     # 32  n-tiles over 2I (mm1 output rows)
NPAIR = NI2 // 2       # 16  (x1, x3) pairs
KI = I // P            # 16  k-tiles over I (mm2 contraction)
ND = D // P            # 8   d-tiles over D (mm2 output rows)

WARMUP_MMS = 17        # scratch 256-col matmuls before real data lands

F32 = mybir.dt.float32
BF16 = mybir.dt.bfloat16
NP_BF16 = ml_dtypes.bfloat16

# exec time of the most recent launch (ns), populated when BASS_TRACE=1
LAST_EXEC_TIME_NS = None

_program_cache = {}


def _chunks_for(C):
    """Split C token-columns into matmul moving-dim chunks (<=512 each)."""
    n_ch = -(-C // 512)
    base = -(-C // (n_ch * 32)) * 32
    chunks = []
    off = 0
    while off < C:
        cn = min(base, C - off)
        chunks.append((off, cn))
        off += cn
    return tuple(chunks)


def _build_program(C):
    chunks = _chunks_for(C)
    PW = C + 4 * P         # piece width: x_k plus pair-0/1 k-slices
    nc = bacc.Bacc(
        "TRN2",
        target_bir_lowering=False,
        debug=False,
        enable_asserts=False,
        num_devices=E,
    )

    # xk[k]: [P, C+512] = [ x_k | wA0_k | wB0_k | wA1_k | wB1_k ] - one
    # contiguous slab per k so each piece is a single fast DMA and the
    # first matmul only waits on piece 0 (~258 KB), not half of x.
    xk_d = nc.dram_tensor("xk", (KD, P, PW), BF16, kind="ExternalInput").ap()
    # w13 pairs 2..15 (pairs 0/1 ride in the pieces)
    w13_d = nc.dram_tensor(
        "w13t", (NPAIR - 2, P, 2 * KD * P), BF16, kind="ExternalInput"
    ).ap()
    w2_d = nc.dram_tensor(
        "w2t", (ND // 2, P, 2 * KI * P), BF16, kind="ExternalInput"
    ).ap()
    out_d = nc.dram_tensor("outT", (ND, P, C), F32, kind="ExternalOutput").ap()

    with tile.TileContext(nc) as tc:
        with (
            tc.tile_pool(name="xg", bufs=1) as xg_pool,
            tc.tile_pool(name="wt", bufs=1) as w_pool,
            tc.tile_pool(name="tmp", bufs=4) as tmp_pool,
            tc.tile_pool(name="ps", bufs=8, space="PSUM") as ps_pool,
        ):
            # ---- PE warmup: scratch matmuls on a memset tile ----
            # gpsimd.memset runs right behind the framework preamble's own
            # Pool memsets (~6.1us), so the first warmup matmul issues
            # ~6.3us - every 213ns of warmup is HAM busy-window credit.
            zt = xg_pool.tile([P, 256], BF16, tag="zt", name="zt")
            nc.gpsimd.memset(zt[:], 0.0)
            for _ in range(WARMUP_MMS):
                psw = ps_pool.tile([P, 256], F32, tag="ps", name="ps")
                nc.tensor.matmul(
                    psw, zt[:, :P], zt[:], start=True, stop=True
                )

            # ---- tiles ----
            pieces = [
                xg_pool.tile([P, PW], BF16, tag=f"pc{k}", name=f"pc{k}")
                for k in range(KD)
            ]
            w13s = {
                pr: w_pool.tile(
                    [P, 2 * KD * P], BF16, tag=f"w13_{pr}", name=f"w13_{pr}"
                )
                for pr in range(2, NPAIR)
            }
            w2s = [
                w_pool.tile(
                    [P, 2 * KI * P], BF16, tag=f"w2_{j}", name=f"w2_{j}"
                )
                for j in range(ND // 2)
            ]

            # ---- all input DMAs on the sync HWDGE ring, need-order ----
            # pair-2's slab is spliced mid-train: it must land before the
            # PE finishes pairs 0/1 (~8 pieces of matmuls), and the later
            # pieces still beat their own consumption comfortably.
            for k in range(4):
                nc.sync.dma_start(pieces[k][:], xk_d[k])
            nc.sync.dma_start(w13s[2][:], w13_d[0])
            for k in range(4, KD):
                nc.sync.dma_start(pieces[k][:], xk_d[k])
            for pr in range(3, NPAIR):
                nc.sync.dma_start(w13s[pr][:], w13_d[pr - 2])
            for j in range(ND // 2):
                nc.sync.dma_start(w2s[j][:], w2_d[j])

            g_tiles = [
                xg_pool.tile([P, C], BF16, tag=f"g{ki}", name=f"g{ki}")
                for ki in range(KI)
            ]

            def x_t(k):
                return pieces[k][:, :C]

            def w01(k, pr, half):
                off = C + (2 * pr + half) * P
                return pieces[k][:, off : off + P]

            # ---- mm1 pairs 0+1: interleaved per-k so each arriving piece
            # feeds four matmuls back-to-back (PE keeps pace with the DMA
            # stream instead of stalling then bursting).
            for c0, cn in chunks:
                ps01 = [
                    ps_pool.tile([P, 512], F32, tag="ps", name="ps")[:, :cn]
                    for _ in range(4)
                ]
                for k in range(KD):
                    for j in range(4):
                        nc.tensor.matmul(
                            ps01[j],
                            w01(k, j // 2, j % 2),
                            x_t(k)[:, c0 : c0 + cn],
                            start=(k == 0),
                            stop=(k == KD - 1),
                        )
                for pr in range(2):
                    s = tmp_pool.tile([P, 512], F32, tag="s", name="s")[:, :cn]
                    nc.scalar.activation(
                        s, ps01[2 * pr], mybir.ActivationFunctionType.Silu
                    )
                    nc.vector.tensor_mul(
                        out=g_tiles[pr][:, c0 : c0 + cn],
                        in0=s,
                        in1=ps01[2 * pr + 1],
                    )

            # ---- mm1 pairs 2..15 from resident slabs ----
            for pr in range(2, NPAIR):
                slab = w13s[pr]
                for c0, cn in chunks:
                    psA = ps_pool.tile([P, 512], F32, tag="ps", name="ps")[:, :cn]
                    psB = ps_pool.tile([P, 512], F32, tag="ps", name="ps")[:, :cn]
                    for half, ps_ in ((0, psA), (1, psB)):
                        for k in range(KD):
                            nc.tensor.matmul(
                                ps_,
                                slab[
                                    :,
                                    half * KD * P + k * P : half * KD * P
                                    + (k + 1) * P,
                                ],
                                x_t(k)[:, c0 : c0 + cn],
                                start=(k == 0),
                                stop=(k == KD - 1),
                            )
                    s = tmp_pool.tile([P, 512], F32, tag="s", name="s")[:, :cn]
                    nc.scalar.activation(s, psA, mybir.ActivationFunctionType.Silu)
                    nc.vector.tensor_mul(
                        out=g_tiles[pr][:, c0 : c0 + cn],
                        in0=s,
                        in1=psB,
                    )

            # ---- mm2: outT[d-block] = sum_ki w2T-tile @ g ----
            # Out DMAs alternate sync/scalar HWDGE rings so the final
            # chunks never queue behind a predecessor's descriptors; the
            # last d-block runs in two column chunks so its copy+DMA
            # mostly overlaps the preceding matmuls.
            for d in range(ND):
                wDD = w2s[d // 2]
                wD = wDD[:, (d % 2) * KI * P : (d % 2 + 1) * KI * P]
                if d == ND - 1 and len(chunks) == 1 and C >= 256:
                    out_chunks = [(0, C - 64), (C - 64, 64)]
                else:
                    out_chunks = chunks
                for ci, (c0, cn) in enumerate(out_chunks):
                    psO = ps_pool.tile([P, 512], F32, tag="ps", name="ps")[:, :cn]
                    for ki in range(KI):
                        nc.tensor.matmul(
                            psO,
                            wD[:, ki * P : (ki + 1) * P],
                            g_tiles[ki][:, c0 : c0 + cn],
                            start=(ki == 0),
                            stop=(ki == KI - 1),
                        )
                    ot = tmp_pool.tile([P, 512], F32, tag="o", name="o")[:, :cn]
                    nc.vector.tensor_copy(ot, psO)
                    if d == ND - 1:
                        eng = nc.scalar if ci == 0 else nc.sync
                    else:
                        eng = nc.scalar if d % 2 == 1 else nc.sync
                    eng.dma_start(out_d[d][:, c0 : c0 + cn], ot)

    nc.compile()
    return nc


def _get_program(C):
    if C not in _program_cache:
        _program_cache[C] = _build_program(C)
    return _program_cache[C]


def _ensure_ntff_hook():
    """Provide antenv.axon_hooks if the image lacks it, so trace=True works."""
    import sys
    import types

    try:
        import antenv.axon_hooks  # noqa: F401

        return
    except ImportError:
        pass
    try:
        import antenv
        from trn_agent_boot.trn_boot import _ntff_profile_via_ctypes

        mod = types.ModuleType("antenv.axon_hooks")
        state = {"hook": None}
        mod.set_axon_ntff_profile_hook = lambda h: state.__setitem__("hook", h)
        mod.get_axon_ntff_profile_hook = lambda: state["hook"]
        sys.modules["antenv.axon_hooks"] = mod
        antenv.axon_hooks = mod
        mod.set_axon_ntff_profile_hook(
            _ntff_profile_via_ctypes("/opt/axon/libaxon_pjrt.so")
        )
    except Exception:
        pass


def kernel(x, w13, w2, expert_indices):
    global LAST_EXEC_TIME_NS
    x = np.asarray(x, dtype=np.float32)
    w13 = np.asarray(w13, dtype=np.float32)
    w2 = np.asarray(w2, dtype=np.float32)
    idx = np.asarray(expert_indices)
    idx32 = idx.astype(np.int64)

    m, d_model = x.shape
    e, two_i, _ = w13.shape
    inter = w2.shape[2]
    topk = idx.shape[1]
    assert (m, d_model, e, two_i, inter, topk) == (M, D, E, 2 * I, I, TOPK)

    # ---- host routing: unique (token, expert) work items per expert ----
    # A token picking the same expert in both slots computes the FFN once;
    # the result is scattered to every matching slot.
    tok_unique = [
        np.unique(np.concatenate([np.nonzero(idx32[:, s] == ei)[0] for s in range(topk)]))
        for ei in range(E)
    ]
    max_cnt = max(len(u) for u in tok_unique)
    C = max(256, int(max_cnt))

    nc = _get_program(C)

    in_maps = []
    for ei in range(E):
        tok_ids = tok_unique[ei]
        cnt = len(tok_ids)

        xg = np.zeros((C, D), dtype=np.float32)
        xg[:cnt] = x[tok_ids]
        xT = np.ascontiguousarray(
            xg.T.reshape(KD, P, C).transpose(1, 0, 2).astype(NP_BF16)
        )                                            # [p, k, c]

        A4 = w13[ei].astype(NP_BF16).reshape(NI2, P, KD, P)   # [n, c, k, p]
        w13t = A4.transpose(0, 3, 2, 1).reshape(NI2, P, KD * P)
        w13p = np.ascontiguousarray(
            np.concatenate([w13t[:NPAIR], w13t[NPAIR:]], axis=2)
        )                                            # [pair, p, 2*KD*P]

        # per-k pieces: [ x_k | wA0_k | wB0_k | wA1_k | wB1_k ]
        xk = np.empty((KD, P, C + 4 * P), dtype=NP_BF16)
        for k in range(KD):
            xk[k, :, :C] = xT[:, k]
            for j, pr, half in ((0, 0, 0), (1, 0, 1), (2, 1, 0), (3, 1, 1)):
                src = w13p[pr][:, half * KD * P + k * P : half * KD * P + (k + 1) * P]
                xk[k, :, C + j * P : C + (j + 1) * P] = src

        B4 = w2[ei].astype(NP_BF16).reshape(ND, P, KI, P)     # [d, c, ki, p]
        w2t = B4.transpose(0, 3, 2, 1).reshape(ND, P, KI * P)
        w2p = np.ascontiguousarray(
            w2t.reshape(ND // 2, 2, P, KI * P).transpose(0, 2, 1, 3).reshape(
                ND // 2, P, 2 * KI * P
            )
        )                                            # [dpair, p, 2*KI*P]

        in_maps.append({"xk": xk, "w13t": w13p[2:], "w2t": w2p})

    trace = bool(os.environ.get("BASS_TRACE"))
    if trace:
        _ensure_ntff_hook()
    res = run_bass_kernel_spmd(nc, in_maps, core_ids=list(range(E)), trace=trace)
    LAST_EXEC_TIME_NS = res.exec_time_ns

    # ---- host scatter: copy each expert's outputs to all matching slots ----
    out = np.empty((M, topk, D), dtype=np.float32)
    for ei in range(E):
        outT = res.results[ei]["outT"].reshape(D, C)
        oe = outT[:, : len(tok_unique[ei])].T        # [cnt, D]
        for s in range(topk):
            sel = np.nonzero(idx32[:, s] == ei)[0]
            out[sel, s] = oe[np.searchsorted(tok_unique[ei], sel)]

    return out


# revision 10
# speedup vs baseline: 1.0101x; 1.0081x over previous
"""Trainium2 Bass kernel for ConditionalFeedForward (MoE routed FFN).

Problem: M=2048 tokens, D=1024, I=2048, E=8 experts, TOPK=2.
out[t, s] = FFN_{e}(x[t]) with e = expert_indices[t, s], where
FFN_e(x) = (silu(x @ w1_e.T) * (x @ w3_e.T)) @ w2_e.T  (w13 = [w1; w3]).

Strategy (expert parallelism, 8 experts -> 8 cores):
 - Host routes (token, slot) pairs to the core owning the selected expert,
   pads each core's token batch to a common capacity C, and transposes
   activations so features live on SBUF partitions and tokens on the free
   dim.  No device collectives needed: the "all-to-all" is a host gather
   and scatter around one SPMD kernel launch.
 - Everything on the PE path is bf16 (PE streams 1 col/cycle for both bf16
   and fp32r, but bf16 halves HBM traffic).  PSUM accumulation is fp32;
   silu*gate runs on ACT+DVE in fp32 and requantizes g to bf16.  fp8 was
   measured at 5.9% rel err end-to-end (gate 2e-2) - dead on arrival.
 - Startup: the critical payload is split into 8 per-k PIECES, each one
   contiguous DRAM tensor slice [x_k | wA0_k | wB0_k | wA1_k | wB1_k], so
   the very first piece (~258 KB) completes ~1.3 us earlier than the old
   two-half layout and pair-0/1 matmuls start as soon as piece 0 lands.
   Pair-2's w13 slab is issued in the middle of the piece train so pair 2
   never stalls.  All input DMAs ride the sync HWDGE ring in need-order.
 - PE warmup: the HAM clock gate holds the PE at 1.2 GHz until it has
   been busy ~3.4us; scratch matmuls on a gpsimd-memset tile run from
   ~6.3us (right after the framework preamble) until real data lands.
 - Tail: the last output d-block runs in two column chunks, and the out
   DMAs alternate between the sync and scalar HWDGE rings so the final
   chunk's DMA never queues behind a predecessor on the same ring.

Measured on the staged inputs: ~95-96 us HW exec (baseline 114.9 us,
previous best 98.7 us), rel l2 err 3.7e-3 vs fp64 (gate 2e-2).
Breakdown: ~80.5us PE stream at the bf16 1 col/cycle roofline (384
matmuls x 497 cols) + ~1us cold-clock penalty + ~2.6us data-arrival
head + ~2us out tail + ~9.5us fixed framework preamble/teardown (the
per-semaphore reset train at end-of-NEFF is paid even by an empty
kernel - measured 14.5us exec for a trivial 2-DMA program).
"""

import os

import numpy as np
import ml_dtypes

import concourse.bass as bass
import concourse.tile as tile
from concourse import bacc, mybir
from concourse.bass_utils import run_bass_kernel_spmd

M, D, I, E, TOPK = 2048, 1024, 2048, 8, 2
P = 128
KD = D // P            # 8   k-tiles over D (mm1 contraction)
NI2 = (2 * I) // P     # 32  n-tiles over 2I (mm1 output rows)
NPAIR = NI2 // 2       # 16  (x1, x3) pairs
KI = I // P            # 16  k-tiles over I (mm2 contraction)
ND = D // P            # 8   d-tiles over D (mm2 output rows)

WARMUP_MMS = 13        # scratch 256-col matmuls before real data lands
NPIECE_PAIRS = 4       # w13 pairs embedded in the startup pieces

F32 = mybir.dt.float32
BF16 = mybir.dt.bfloat16
NP_BF16 = ml_dtypes.bfloat16

# exec time of the most recent launch (ns), populated when BASS_TRACE=1
LAST_EXEC_TIME_NS = None

_program_cache = {}


def _chunks_for(C):
    """Split C token-columns into matmul moving-dim chunks (<=512 each)."""
    n_ch = -(-C // 512)
    base = -(-C // (n_ch * 32)) * 32
    chunks = []
    off = 0
    while off < C:
        cn = min(base, C - off)
        chunks.append((off, cn))
        off += cn
    return tuple(chunks)


def _build_program(C):
    chunks = _chunks_for(C)
    PW = C + 2 * NPIECE_PAIRS * P  # piece width: x_k plus pair-0..3 k-slices
    nc = bacc.Bacc(
        "TRN2",
        target_bir_lowering=False,
        debug=False,
        enable_asserts=False,
        num_devices=E,
    )

    # xk[k]: [P, C+1024] = [ x_k | wA0_k | wB0_k | ... | wA3_k | wB3_k ] -
    # one contiguous slab per k so each piece is a single fast DMA and the
    # first matmul only waits on piece 0 (~390 KB), not half of x.  Four
    # pairs ride in the pieces so the PE consumes 8 matmuls per piece
    # (~1.7us) while pieces arrive every ~1.2us - no arrival stalls - and
    # the first w13 slab (pair 4) has ~8us of delivery slack.
    xk_d = nc.dram_tensor("xk", (KD, P, PW), BF16, kind="ExternalInput").ap()
    # w13 pairs 4..15 (pairs 0-3 ride in the pieces)
    w13_d = nc.dram_tensor(
        "w13t", (NPAIR - NPIECE_PAIRS, P, 2 * KD * P), BF16, kind="ExternalInput"
    ).ap()
    w2_d = nc.dram_tensor(
        "w2t", (ND // 2, P, 2 * KI * P), BF16, kind="ExternalInput"
    ).ap()
    out_d = nc.dram_tensor("outT", (ND, P, C), F32, kind="ExternalOutput").ap()

    with tile.TileContext(nc) as tc:
        with (
            tc.tile_pool(name="xg", bufs=1) as xg_pool,
            tc.tile_pool(name="wt", bufs=1) as w_pool,
            tc.tile_pool(name="tmp", bufs=4) as tmp_pool,
            tc.tile_pool(name="ps", bufs=8, space="PSUM") as ps_pool,
        ):
            # ---- PE warmup: scratch matmuls on a memset tile ----
            # gpsimd.memset runs right behind the framework preamble's own
            # Pool memsets (~6.1us), so the first warmup matmul issues
            # ~6.3us - every 213ns of warmup is HAM busy-window credit.
            zt = xg_pool.tile([P, 256], BF16, tag="zt", name="zt")
            nc.gpsimd.memset(zt[:], 0.0)
            for _ in range(WARMUP_MMS):
                psw = ps_pool.tile([P, 256], F32, tag="ps", name="ps")
                nc.tensor.matmul(
                    psw, zt[:, :P], zt[:], start=True, stop=True
                )

            # ---- tiles ----
            pieces = [
                xg_pool.tile([P, PW], BF16, tag=f"pc{k}", name=f"pc{k}")
                for k in range(KD)
            ]
            w13s = {
                pr: w_pool.tile(
                    [P, 2 * KD * P], BF16, tag=f"w13_{pr}", name=f"w13_{pr}"
                )
                for pr in range(NPIECE_PAIRS, NPAIR)
            }
            w2s = [
                w_pool.tile(
                    [P, 2 * KI * P], BF16, tag=f"w2_{j}", name=f"w2_{j}"
                )
                for j in range(ND // 2)
            ]

            # ---- all input DMAs on the sync HWDGE ring, need-order ----
            # pair-4's slab lands ~16us while the PE only reaches pair 4
            # at ~25us (after 64 piece matmuls) - ample slack.
            for k in range(KD):
                nc.sync.dma_start(pieces[k][:], xk_d[k])
            for pr in range(NPIECE_PAIRS, NPAIR):
                nc.sync.dma_start(w13s[pr][:], w13_d[pr - NPIECE_PAIRS])
            for j in range(ND // 2):
                nc.sync.dma_start(w2s[j][:], w2_d[j])

            g_tiles = [
                xg_pool.tile([P, C], BF16, tag=f"g{ki}", name=f"g{ki}")
                for ki in range(KI)
            ]

            def x_t(k):
                return pieces[k][:, :C]

            def w01(k, pr, half):
                off = C + (2 * pr + half) * P
                return pieces[k][:, off : off + P]

            # ---- mm1 pairs 0..3: interleaved per-k so each arriving piece
            # feeds eight matmuls back-to-back (PE keeps pace with the DMA
            # stream instead of stalling then bursting).  Uses all 8 PSUM
            # banks as concurrent accumulation groups.
            for c0, cn in chunks:
                ps01 = [
                    ps_pool.tile([P, 512], F32, tag="ps", name="ps")[:, :cn]
                    for _ in range(2 * NPIECE_PAIRS)
                ]
                for k in range(KD):
                    for j in range(2 * NPIECE_PAIRS):
                        nc.tensor.matmul(
                            ps01[j],
                            w01(k, j // 2, j % 2),
                            x_t(k)[:, c0 : c0 + cn],
                            start=(k == 0),
                            stop=(k == KD - 1),
                        )
                for pr in range(NPIECE_PAIRS):
                    s = tmp_pool.tile([P, 512], F32, tag="s", name="s")[:, :cn]
                    nc.scalar.activation(
                        s, ps01[2 * pr], mybir.ActivationFunctionType.Silu
                    )
                    nc.vector.tensor_mul(
                        out=g_tiles[pr][:, c0 : c0 + cn],
                        in0=s,
                        in1=ps01[2 * pr + 1],
                    )

            # ---- mm1 pairs 4..15 from resident slabs ----
            for pr in range(NPIECE_PAIRS, NPAIR):
                slab = w13s[pr]
                for c0, cn in chunks:
                    psA = ps_pool.tile([P, 512], F32, tag="ps", name="ps")[:, :cn]
                    psB = ps_pool.tile([P, 512], F32, tag="ps", name="ps")[:, :cn]
                    for half, ps_ in ((0, psA), (1, psB)):
                        for k in range(KD):
                            nc.tensor.matmul(
                                ps_,
                                slab[
                                    :,
                                    half * KD * P + k * P : half * KD * P
                                    + (k + 1) * P,
                                ],
                                x_t(k)[:, c0 : c0 + cn],
                                start=(k == 0),
                                stop=(k == KD - 1),
                            )
                    s = tmp_pool.tile([P, 512], F32, tag="s", name="s")[:, :cn]
                    nc.scalar.activation(s, psA, mybir.ActivationFunctionType.Silu)
                    nc.vector.tensor_mul(
                        out=g_tiles[pr][:, c0 : c0 + cn],
                        in0=s,
                        in1=psB,
                    )

            # ---- mm2: outT[d-block] = sum_ki w2T-tile @ g ----
            # Out DMAs alternate sync/scalar HWDGE rings so the final
            # chunks never queue behind a predecessor's descriptors; the
            # last d-block runs in two column chunks so its copy+DMA
            # mostly overlaps the preceding matmuls.
            for d in range(ND):
                wDD = w2s[d // 2]
                wD = wDD[:, (d % 2) * KI * P : (d % 2 + 1) * KI * P]
                if d == ND - 1 and len(chunks) == 1 and C >= 256:
                    out_chunks = [(0, C - 64), (C - 64, 64)]
                else:
                    out_chunks = chunks
                for ci, (c0, cn) in enumerate(out_chunks):
                    psO = ps_pool.tile([P, 512], F32, tag="ps", name="ps")[:, :cn]
                    for ki in range(KI):
                        nc.tensor.matmul(
                            psO,
                            wD[:, ki * P : (ki + 1) * P],
                            g_tiles[ki][:, c0 : c0 + cn],
                            start=(ki == 0),
                            stop=(ki == KI - 1),
                        )
                    ot = tmp_pool.tile([P, 512], F32, tag="o", name="o")[:, :cn]
                    # last d-block: the two chunks' copy+DMA chains run on
                    # disjoint engine pairs (DVE+sync / ACT+scalar) so the
                    # final 64-col chunk never queues behind the 433-col
                    # chunk's copy or descriptors.
                    if d == ND - 1 and len(out_chunks) > 1 and ci > 0:
                        nc.scalar.copy(ot, psO)
                        nc.scalar.dma_start(out_d[d][:, c0 : c0 + cn], ot)
                    else:
                        nc.vector.tensor_copy(ot, psO)
                        eng = nc.scalar if (d % 2 == 1 and d != ND - 1) else nc.sync
                        eng.dma_start(out_d[d][:, c0 : c0 + cn], ot)

    nc.compile()
    return nc


def _get_program(C):
    if C not in _program_cache:
        _program_cache[C] = _build_program(C)
    return _program_cache[C]


def _ensure_ntff_hook():
    """Provide antenv.axon_hooks if the image lacks it, so trace=True works."""
    import sys
    import types

    try:
        import antenv.axon_hooks  # noqa: F401

        return
    except ImportError:
        pass
    try:
        import antenv
        from trn_agent_boot.trn_boot import _ntff_profile_via_ctypes

        mod = types.ModuleType("antenv.axon_hooks")
        state = {"hook": None}
        mod.set_axon_ntff_profile_hook = lambda h: state.__setitem__("hook", h)
        mod.get_axon_ntff_profile_hook = lambda: state["hook"]
        sys.modules["antenv.axon_hooks"] = mod
        antenv.axon_hooks = mod
        mod.set_axon_ntff_profile_hook(
            _ntff_profile_via_ctypes("/opt/axon/libaxon_pjrt.so")
        )
    except Exception:
        pass


def kernel(x, w13, w2, expert_indices):
    global LAST_EXEC_TIME_NS
    x = np.asarray(x, dtype=np.float32)
    w13 = np.asarray(w13, dtype=np.float32)
    w2 = np.asarray(w2, dtype=np.float32)
    idx = np.asarray(expert_indices)
    idx32 = idx.astype(np.int64)

    m, d_model = x.shape
    e, two_i, _ = w13.shape
    inter = w2.shape[2]
    topk = idx.shape[1]
    assert (m, d_model, e, two_i, inter, topk) == (M, D, E, 2 * I, I, TOPK)

    # ---- host routing: unique (token, expert) work items per expert ----
    # A token picking the same expert in both slots computes the FFN once;
    # the result is scattered to every matching slot.
    tok_unique = [
        np.unique(np.concatenate([np.nonzero(idx32[:, s] == ei)[0] for s in range(topk)]))
        for ei in range(E)
    ]
    max_cnt = max(len(u) for u in tok_unique)
    C = max(256, int(max_cnt))

    nc = _get_program(C)

    in_maps = []
    for ei in range(E):
        tok_ids = tok_unique[ei]
        cnt = len(tok_ids)

        xg = np.zeros((C, D), dtype=np.float32)
        xg[:cnt] = x[tok_ids]
        xT = np.ascontiguousarray(
            xg.T.reshape(KD, P, C).transpose(1, 0, 2).astype(NP_BF16)
        )                                            # [p, k, c]

        A4 = w13[ei].astype(NP_BF16).reshape(NI2, P, KD, P)   # [n, c, k, p]
        w13t = A4.transpose(0, 3, 2, 1).reshape(NI2, P, KD * P)
        w13p = np.ascontiguousarray(
            np.concatenate([w13t[:NPAIR], w13t[NPAIR:]], axis=2)
        )                                            # [pair, p, 2*KD*P]

        # per-k pieces: [ x_k | wA0_k | wB0_k | ... | wA3_k | wB3_k ]
        xk = np.empty((KD, P, C + 2 * NPIECE_PAIRS * P), dtype=NP_BF16)
        for k in range(KD):
            xk[k, :, :C] = xT[:, k]
            for j in range(2 * NPIECE_PAIRS):
                pr, half = j // 2, j % 2
                src = w13p[pr][:, half * KD * P + k * P : half * KD * P + (k + 1) * P]
                xk[k, :, C + j * P : C + (j + 1) * P] = src

        B4 = w2[ei].astype(NP_BF16).reshape(ND, P, KI, P)     # [d, c, ki, p]
        w2t = B4.transpose(0, 3, 2, 1).reshape(ND, P, KI * P)
        w2p = np.ascontiguousarray(
            w2t.reshape(ND // 2, 2, P, KI * P).transpose(0, 2, 1, 3).reshape(
                ND // 2, P, 2 * KI * P
            )
        )                                            # [dpair, p, 2*KI*P]

        in_maps.append({"xk": xk, "w13t": w13p[NPIECE_PAIRS:], "w2t": w2p})

    trace = bool(os.environ.get("BASS_TRACE"))
    if trace:
        _ensure_ntff_hook()
    res = run_bass_kernel_spmd(nc, in_maps, core_ids=list(range(E)), trace=trace)
    LAST_EXEC_TIME_NS = res.exec_time_ns

    # ---- host scatter: copy each expert's outputs to all matching slots ----
    out = np.empty((M, topk, D), dtype=np.float32)
    for ei in range(E):
        outT = res.results[ei]["outT"].reshape(D, C)
        oe = outT[:, : len(tok_unique[ei])].T        # [cnt, D]
        for s in range(topk):
            sel = np.nonzero(idx32[:, s] == ei)[0]
            out[sel, s] = oe[np.searchsorted(tok_unique[ei], sel)]

    return out
